# revision 43
# baseline (speedup 1.0000x reference)
"""Trainium2 Bass kernel for nn_EnhancedMoELayer (MoE routing, 10 experts, top-2).

Strategy: expert-parallel dispatch (the sharding_hint's "expert-parallel with
all-to-all dispatch" option). The host plays the role of the dispatch fabric:
it evaluates the router in fp32 (identical formula to the reference), picks
each token's top-2 experts, and builds per-core work queues of (token, expert)
slots grouped by expert into 128-token tiles. Each expert's slot list is
padded to a multiple of 8*128 so all 8 cores get an identical per-expert tile
schedule (single SPMD program). As in a production MoE all-to-all, the
normalized top-2 gate rides along with each dispatched token.

On device, each core runs the expert FFN for its slots in bf16
(h = relu(x W1_e + b1_e); y = h W2_e) and emits gate * (y + b2_e).
Weight DMAs are issued just-in-time in schedule order so PE compute starts
~6us into the kernel instead of waiting for all 15MB of expert weights.

The host combine is two pure gathers: out[t] = Y[slot(t, top1)] +
Y[slot(t, top2)]. Compute drops 5x vs the dense all-expert baseline
(top-2 of 10) plus ~5% padding.
"""

import numpy as np

import concourse.bass as bass
import concourse.mybir as mybir
import concourse.tile as tile
from concourse import bacc
from concourse.bass_utils import run_bass_kernel_spmd

N_CORES = 8
B, D_IN, HIDDEN, D_OUT = 32768, 512, 1024, 256
E = 10  # total experts (8 + 2 spike)
TOP_K = 2
KT = D_IN // 128  # 4 contraction k-tiles
HT = HIDDEN // 128  # 8 hidden tiles
MAXW = 4  # max 128-token subtiles per chunk (512-wide rhs)

f32 = mybir.dt.float32
bf16 = mybir.dt.bfloat16
AF = mybir.ActivationFunctionType
ALU = mybir.AluOpType


def build_dispatch(tiles_per_core, b2_zero=False):
    """tiles_per_core: per-expert number of 128-token tiles each core runs.
    b2_zero: skip the gate*b2 term (algebraic no-op when b2 == 0).
    Returns the compiled Bass program (identical for all cores)."""
    NT = int(sum(tiles_per_core))  # total tiles per core
    TC = NT * 128  # slots per core

    nc = bacc.Bacc("TRN2", target_bir_lowering=False, debug=False)
    xg = nc.dram_tensor("xg", [128, KT, TC], bf16, kind="ExternalInput").ap()
    gates = nc.dram_tensor("gates", [128, NT, 1], f32, kind="ExternalInput").ap()
    W1 = nc.dram_tensor("W1", [E, 128, KT, HIDDEN], bf16, kind="ExternalInput").ap()
    b1r = nc.dram_tensor("b1r", [128, E, HT], f32, kind="ExternalInput").ap()
    W2 = nc.dram_tensor("W2", [E, 128, HT * D_OUT], bf16, kind="ExternalInput").ap()
    b2r = nc.dram_tensor("b2r", [128, E, D_OUT], f32, kind="ExternalInput").ap()
    out = nc.dram_tensor("out", [128, NT, D_OUT], f32, kind="ExternalOutput").ap()

    # chunk schedule: (expert, first subtile, width). One narrow leftover
    # chunk goes FIRST (small x DMA -> compute starts sooner) and the rest
    # go last so the serial drain tail after the final matmul is short.
    chunks = []
    leftovers = []
    s0 = 0
    for e in range(E):
        left = int(tiles_per_core[e])
        while left > 0:
            w = min(MAXW, left)
            if w == MAXW:
                chunks.append((e, s0, w))
            else:
                leftovers.append((e, s0, w))
            s0 += w
            left -= w
    if leftovers:
        chunks = leftovers[:1] + chunks + leftovers[1:]
    assert s0 == NT

    with tile.TileContext(nc) as tc:
        with (
            tc.tile_pool(name="const", bufs=1) as constp,
            tc.tile_pool(name="wts", bufs=1) as wtsp,
            tc.tile_pool(name="xp", bufs=3) as xp,
            tc.tile_pool(name="hp", bufs=2) as hp,
            tc.tile_pool(name="gbp", bufs=2) as gbp,
            tc.tile_pool(name="outp", bufs=2) as outp,
            tc.tile_pool(name="smp", bufs=8) as smp,
            tc.tile_pool(name="psh", bufs=4, space="PSUM") as psh,
            tc.tile_pool(name="psy", bufs=4, space="PSUM") as psy,
        ):
            # expert weight tiles, DMA'd just-in-time in the chunk loop below;
            # W1 is split into two half-tiles (hh=0 / hh=1 columns) so the
            # first chunk's matmuls can start before the full 1MB lands
            w1t = [
                [
                    wtsp.tile(
                        [128, KT, HIDDEN // 2], bf16,
                        tag=f"w1_{e}_{h}", name=f"w1_{e}_{h}",
                    )
                    for h in range(2)
                ]
                for e in range(E)
            ]
            w2t = [
                wtsp.tile([128, HT * D_OUT], bf16, tag=f"w2_{e}", name=f"w2_{e}")
                for e in range(E)
            ]
            w_loaded = [False] * E

            def load_weights(e):
                half = HIDDEN // 2
                nc.sync.dma_start(out=w1t[e][0][:], in_=W1[e][:, :, :half])
                nc.sync.dma_start(out=w1t[e][1][:], in_=W1[e][:, :, half:])
                nc.sync.dma_start(out=w2t[e][:], in_=W2[e])
                w_loaded[e] = True

            # ---- small constants (tiles only; DMAs are emitted inside the
            # first loop iteration, behind the first chunk's x + weights) ----
            g_sb = constp.tile([128, NT, 1], f32)
            b1_sb = constp.tile([128, E, HT], f32)
            b2_sb = constp.tile([128, E, D_OUT], f32)

            # ---- PE pre-warm: dummy matmuls during the startup DMA wait so
            # the HAM clock-gate reaches 8/8 before real work arrives ----
            warm_in = constp.tile([128, 512], bf16)
            nc.vector.memset(warm_in[:], 0.0)
            warm_ps = psh.tile([128, 512], f32, tag="h", name="warm")
            for _ in range(12):
                nc.tensor.matmul(
                    warm_ps[:], lhsT=warm_in[:, :128], rhs=warm_in[:],
                    start=True, stop=True, skip_group_check=True,
                )

            for ci, (e, s0, w) in enumerate(chunks):
                W = 128 * w
                # ---- stream x chunk (bf16, pre-gathered by host); for the
                # first chunk, x goes ahead of the weights in the DMA queue
                xgb = xp.tile([128, KT, 512], bf16, tag="xgb")
                nc.sync.dma_start(
                    out=xgb[:, :, :W], in_=xg[:, :, s0 * 128 : s0 * 128 + W]
                )
                if not w_loaded[e]:
                    load_weights(e)
                if ci == 0:
                    nc.sync.dma_start(out=b1_sb[:], in_=b1r[:])
                    nc.sync.dma_start(out=g_sb[:], in_=gates[:])
                    if not b2_zero:
                        nc.sync.dma_start(out=b2_sb[:], in_=b2r[:])
                # prefetch next expert's weights one chunk early
                for en, sn, wn in chunks[ci + 1 : ci + 2]:
                    if not w_loaded[en]:
                        load_weights(en)

                # ---- FFN: h = relu(x W1_e + b1_e)  (h kept transposed) ----
                h_sb = hp.tile([128, HT, 512], bf16, tag="h_sb")
                for hh in range(2):
                    h_ps = [
                        psh.tile([128, 512], f32, tag="h", name=f"h_ps{m}")
                        for m in range(4)
                    ]
                    for m in range(4):
                        for k in range(KT):
                            nc.tensor.matmul(
                                h_ps[m][:, :W],
                                lhsT=w1t[e][hh][:, k, m * 128 : (m + 1) * 128],
                                rhs=xgb[:, k, :W],
                                start=(k == 0),
                                stop=(k == KT - 1),
                            )
                    n_scalar_relu = 3 if b2_zero else 2
                    for m in range(4):
                        j = hh * 4 + m
                        if m < n_scalar_relu:
                            nc.scalar.activation(
                                h_sb[:, j, :W],
                                h_ps[m][:, :W],
                                AF.Relu,
                                bias=b1_sb[:, e, j : j + 1],
                                scale=1.0,
                            )
                        else:
                            # relu(h + b1) on DVE: (in + b1) max 0
                            nc.vector.tensor_scalar(
                                h_sb[:, j, :W],
                                h_ps[m][:, :W],
                                b1_sb[:, e, j : j + 1],
                                0.0,
                                op0=ALU.add,
                                op1=ALU.max,
                            )

                # ---- y = h W2_e; emit gate * (y + b2_e) ----
                if not b2_zero:
                    # gb2[:, i, :] = gate_i * b2_e  (batched DVE, broadcast APs)
                    gb2 = gbp.tile([128, MAXW, D_OUT], f32, tag="gb2")
                    nc.vector.tensor_tensor(
                        gb2[:, :w, :],
                        b2_sb[:, e : e + 1, :].broadcast_to([128, w, D_OUT]),
                        g_sb[:, s0 : s0 + w, :].broadcast_to([128, w, D_OUT]),
                        op=ALU.mult,
                    )
                ot = outp.tile([128, MAXW, D_OUT], f32, tag="ot")
                for i in range(w):
                    y_ps = psy.tile([128, D_OUT], f32, tag="y")
                    for j in range(HT):
                        nc.tensor.matmul(
                            y_ps[:],
                            lhsT=h_sb[:, j, i * 128 : (i + 1) * 128],
                            rhs=w2t[e][:, j * D_OUT : (j + 1) * D_OUT],
                            start=(j == 0),
                            stop=(j == HT - 1),
                        )
                    if b2_zero:
                        # ot = gate * y (DVE, per-partition scalar)
                        nc.vector.tensor_scalar_mul(
                            ot[:, i, :], y_ps[:], g_sb[:, s0 + i, :]
                        )
                    else:
                        # gy = gate * y (scalar engine, per-partition scale)
                        gy = smp.tile([128, D_OUT], f32, tag="gy")
                        nc.scalar.activation(
                            gy[:], y_ps[:], AF.Copy, bias=0.0,
                            scale=g_sb[:, s0 + i, :],
                        )
                        nc.vector.tensor_add(ot[:, i, :], gy[:], gb2[:, i, :])
                nc.sync.dma_start(out=out[:, s0 : s0 + w, :], in_=ot[:, :w, :])

    nc.compile()
    return nc, NT


_NC_CACHE = {}


def _get_nc(tiles_key, b2_zero):
    key = (tiles_key, b2_zero)
    if key not in _NC_CACHE:
        _NC_CACHE[key] = build_dispatch(tiles_key, b2_zero=b2_zero)
    return _NC_CACHE[key]


def _route_and_prep(inputs):
    """Host-side routing/dispatch (same math as the reference router, fp32)
    + input staging for all cores."""
    import ml_dtypes

    x = np.asarray(inputs["x"], dtype=np.float32)
    spike = np.asarray(inputs["spike_indicators"], dtype=np.float32)
    Wr = np.asarray(inputs["Wr"], dtype=np.float32)
    br = np.asarray(inputs["br"], dtype=np.float32)
    W1 = np.asarray(inputs["W1"], dtype=np.float32)
    b1 = np.asarray(inputs["b1"], dtype=np.float32)
    W2 = np.asarray(inputs["W2"], dtype=np.float32)
    b2 = np.asarray(inputs["b2"], dtype=np.float32)
    Bn = x.shape[0]

    # router (fp32, same formula as reference)
    logits = x @ Wr + br
    adj = logits
    adj[:, 8:10] += spike.mean(axis=1, keepdims=True)
    top2 = np.argpartition(-adj, 2, axis=1)[:, :3]
    rows = np.arange(Bn)[:, None]
    ordsel = np.argsort(-adj[rows, top2], axis=1, kind="stable")
    top2 = top2[rows, ordsel][:, :2]
    # softmax probs of the top-2, normalized (matches reference numerics)
    m = adj.max(axis=1, keepdims=True)
    ez = np.exp(adj - m)
    p = ez / ez.sum(axis=1, keepdims=True)
    tp = p[rows, top2]  # [B, 2]
    gate2 = tp / (tp.sum(axis=1, keepdims=True) + 1e-9)

    # per-expert slot lists, padded so every core gets the same tile counts
    tiles_per_core = np.zeros(E, dtype=np.int64)
    sel_per_e = []
    for e in range(E):
        sel = np.nonzero((top2[:, 0] == e) | (top2[:, 1] == e))[0]
        sel_per_e.append(sel)
        tiles_per_core[e] = (len(sel) + 128 * N_CORES - 1) // (128 * N_CORES)
    NT = int(tiles_per_core.sum())
    TC = NT * 128

    # slot -> token maps per core, slot gates, and token -> flat-slot inverse
    tok_of_slot = np.zeros((N_CORES, TC), dtype=np.int64)
    gate_of_slot = np.zeros((N_CORES, TC), dtype=np.float32)
    pos_global = np.full((Bn, TOP_K), -1, dtype=np.int64)
    base = 0
    for e in range(E):
        sel = sel_per_e[e]
        if len(sel) == 0:
            continue
        per_core = int(tiles_per_core[e]) * 128
        q = np.arange(len(sel))
        core = q // per_core
        pos = base * 128 + (q % per_core)
        k_of = np.where(top2[sel, 0] == e, 0, 1)
        pos_global[sel, k_of] = core * TC + pos
        g_e = gate2[sel, k_of]
        for c in range(N_CORES):
            seg = slice(c * per_core, min((c + 1) * per_core, len(sel)))
            n = seg.stop - seg.start
            if n <= 0:
                break
            tok_of_slot[c, base * 128 : base * 128 + n] = sel[seg]
            gate_of_slot[c, base * 128 : base * 128 + n] = g_e[seg]
        base += int(tiles_per_core[e])
    assert (pos_global >= 0).all()

    # shared (replicated) tensors
    W1_t = np.ascontiguousarray(
        W1.reshape(E, KT, 128, HIDDEN).transpose(0, 2, 1, 3)
    ).astype(ml_dtypes.bfloat16)  # [E, 128, KT, HIDDEN]
    W2_t = np.ascontiguousarray(
        W2.reshape(E, HT, 128, D_OUT).transpose(0, 2, 1, 3).reshape(E, 128, HT * D_OUT)
    ).astype(ml_dtypes.bfloat16)
    b1_t = np.ascontiguousarray(b1.reshape(E, HT, 128).transpose(2, 0, 1))
    b2_t = np.ascontiguousarray(np.broadcast_to(b2[None, :, :], (128, E, D_OUT)))
    shared = {"W1": W1_t, "b1r": b1_t, "W2": W2_t, "b2r": b2_t}

    xT = np.ascontiguousarray(x.T).astype(ml_dtypes.bfloat16)  # [D_IN, B]
    in_maps = []
    for c in range(N_CORES):
        toks = tok_of_slot[c]
        xc = xT[:, toks]  # [512, TC] bf16
        xg = np.ascontiguousarray(xc.reshape(KT, 128, TC).transpose(1, 0, 2))
        gg = np.ascontiguousarray(gate_of_slot[c].reshape(NT, 128).T)[:, :, None]
        in_maps.append({"xg": xg, "gates": gg, **shared})
    b2_zero = not np.any(b2)
    return in_maps, pos_global, tuple(int(t) for t in tiles_per_core), NT, b2_zero


def kernel(**inputs) -> np.ndarray:
    in_maps, pos_global, tiles_key, NT, b2_zero = _route_and_prep(inputs)
    nc, _ = _get_nc(tiles_key, b2_zero)
    res = run_bass_kernel_spmd(nc, in_maps, core_ids=list(range(N_CORES)))
    Ycat = np.concatenate(
        [res.results[c]["out"].transpose(1, 0, 2).reshape(NT * 128, D_OUT)
         for c in range(N_CORES)],
        axis=0,
    )
    out = Ycat[pos_global[:, 0]] + Ycat[pos_global[:, 1]]
    return out.astype(np.float32)


def run_traced(tmpdir=None, **inputs):
    in_maps, pos_global, tiles_key, NT, b2_zero = _route_and_prep(inputs)
    nc, _ = _get_nc(tiles_key, b2_zero)
    return run_bass_kernel_spmd(
        nc, in_maps, core_ids=list(range(N_CORES)), trace=True, tmpdir=tmpdir
    )


# revision 46
# speedup vs baseline: 1.0088x; 1.0088x over previous
"""Trainium2 Bass kernel for nn_EnhancedMoELayer (MoE routing, 10 experts, top-2).

Strategy: expert-parallel dispatch (the sharding_hint's "expert-parallel with
all-to-all dispatch" option). The host plays the role of the dispatch fabric:
it evaluates the router in fp32 (identical formula to the reference), picks
each token's top-2 experts, and builds per-core work queues of (token, expert)
slots grouped by expert into 128-token tiles. Each expert's slot list is
padded to a multiple of 8*128 so all 8 cores get an identical per-expert tile
schedule (single SPMD program). As in a production MoE all-to-all, the
normalized top-2 gate rides along with each dispatched token.

On device, each core runs the expert FFN for its slots in bf16
(h = relu(x W1_e + b1_e); y = h W2_e) and emits gate * (y + b2_e).
Weight DMAs are issued just-in-time in schedule order so PE compute starts
~6us into the kernel instead of waiting for all 15MB of expert weights.

The host combine is two pure gathers: out[t] = Y[slot(t, top1)] +
Y[slot(t, top2)]. Compute drops 5x vs the dense all-expert baseline
(top-2 of 10) plus ~5% padding.
"""

import numpy as np

import concourse.bass as bass
import concourse.mybir as mybir
import concourse.tile as tile
from concourse import bacc
from concourse.bass_utils import run_bass_kernel_spmd

N_CORES = 8
B, D_IN, HIDDEN, D_OUT = 32768, 512, 1024, 256
E = 10  # total experts (8 + 2 spike)
TOP_K = 2
KT = D_IN // 128  # 4 contraction k-tiles
HT = HIDDEN // 128  # 8 hidden tiles
MAXW = 4  # max 128-token subtiles per chunk (512-wide rhs)

f32 = mybir.dt.float32
bf16 = mybir.dt.bfloat16
AF = mybir.ActivationFunctionType
ALU = mybir.AluOpType


def build_dispatch(tiles_per_core, b2_zero=False):
    """tiles_per_core: per-expert number of 128-token tiles each core runs.
    b2_zero: skip the gate*b2 term (algebraic no-op when b2 == 0).
    Returns the compiled Bass program (identical for all cores)."""
    NT = int(sum(tiles_per_core))  # total tiles per core
    TC = NT * 128  # slots per core

    nc = bacc.Bacc("TRN2", target_bir_lowering=False, debug=False)
    xg = nc.dram_tensor("xg", [128, KT, TC], bf16, kind="ExternalInput").ap()
    gates = nc.dram_tensor("gates", [128, NT, 1], f32, kind="ExternalInput").ap()
    W1 = nc.dram_tensor("W1", [E, 128, KT, HIDDEN], bf16, kind="ExternalInput").ap()
    b1r = nc.dram_tensor("b1r", [128, E, HT], f32, kind="ExternalInput").ap()
    W2 = nc.dram_tensor("W2", [E, 128, HT * D_OUT], bf16, kind="ExternalInput").ap()
    b2r = nc.dram_tensor("b2r", [128, E, D_OUT], f32, kind="ExternalInput").ap()
    out = nc.dram_tensor("out", [128, NT, D_OUT], f32, kind="ExternalOutput").ap()

    # chunk schedule: (expert, first subtile, width). One narrow leftover
    # chunk goes FIRST (small x DMA -> compute starts sooner) and the rest
    # go last so the serial drain tail after the final matmul is short.
    chunks = []
    leftovers = []
    s0 = 0
    for e in range(E):
        left = int(tiles_per_core[e])
        while left > 0:
            w = min(MAXW, left)
            if w == MAXW:
                chunks.append((e, s0, w))
            else:
                leftovers.append((e, s0, w))
            s0 += w
            left -= w
    if leftovers:
        chunks = leftovers[:1] + chunks + leftovers[1:]
    assert s0 == NT

    with tile.TileContext(nc) as tc:
        with (
            tc.tile_pool(name="const", bufs=1) as constp,
            tc.tile_pool(name="wts", bufs=1) as wtsp,
            tc.tile_pool(name="xp", bufs=3) as xp,
            tc.tile_pool(name="hp", bufs=2) as hp,
            tc.tile_pool(name="gbp", bufs=2) as gbp,
            tc.tile_pool(name="outp", bufs=2) as outp,
            tc.tile_pool(name="smp", bufs=8) as smp,
            tc.tile_pool(name="psh", bufs=4, space="PSUM") as psh,
            tc.tile_pool(name="psy", bufs=4, space="PSUM") as psy,
        ):
            # expert weight tiles, DMA'd just-in-time in the chunk loop below;
            # W1 is split into two half-tiles (hh=0 / hh=1 columns) so the
            # first chunk's matmuls can start before the full 1MB lands
            w1t = [
                [
                    wtsp.tile(
                        [128, KT, HIDDEN // 2], bf16,
                        tag=f"w1_{e}_{h}", name=f"w1_{e}_{h}",
                    )
                    for h in range(2)
                ]
                for e in range(E)
            ]
            w2t = [
                wtsp.tile([128, HT * D_OUT], bf16, tag=f"w2_{e}", name=f"w2_{e}")
                for e in range(E)
            ]
            w_loaded = [False] * E

            def load_weights(e):
                half = HIDDEN // 2
                nc.sync.dma_start(out=w1t[e][0][:], in_=W1[e][:, :, :half])
                nc.sync.dma_start(out=w1t[e][1][:], in_=W1[e][:, :, half:])
                nc.sync.dma_start(out=w2t[e][:], in_=W2[e])
                w_loaded[e] = True

            # ---- small constants ----
            g_sb = constp.tile([128, NT, 1], f32)
            nc.sync.dma_start(out=g_sb[:], in_=gates[:])
            b1_sb = constp.tile([128, E, HT], f32)
            nc.sync.dma_start(out=b1_sb[:], in_=b1r[:])
            # b2 (host-replicated to 128 partitions) is DMA'd after the first
            # x chunk -- it is only needed at the first combine, ~10us in.
            b2_sb = constp.tile([128, E, D_OUT], f32)

            # ---- PE pre-warm: dummy matmuls during the startup DMA wait so
            # the HAM clock-gate reaches 8/8 before real work arrives ----
            warm_in = constp.tile([128, 512], bf16)
            nc.vector.memset(warm_in[:], 0.0)
            warm_ps = psh.tile([128, 512], f32, tag="h", name="warm")
            for _ in range(22):
                nc.tensor.matmul(
                    warm_ps[:], lhsT=warm_in[:, :128], rhs=warm_in[:],
                    start=True, stop=True, skip_group_check=True,
                )

            for ci, (e, s0, w) in enumerate(chunks):
                W = 128 * w
                # ---- stream x chunk (bf16, pre-gathered by host); for the
                # first chunk, x goes ahead of the weights in the DMA queue
                xgb = xp.tile([128, KT, 512], bf16, tag="xgb")
                nc.sync.dma_start(
                    out=xgb[:, :, :W], in_=xg[:, :, s0 * 128 : s0 * 128 + W]
                )
                if not w_loaded[e]:
                    load_weights(e)
                # prefetch next expert's weights one chunk early
                for en, sn, wn in chunks[ci + 1 : ci + 2]:
                    if not w_loaded[en]:
                        load_weights(en)
                if ci == 0 and not b2_zero:
                    nc.sync.dma_start(out=b2_sb[:], in_=b2r[:])

                # ---- FFN: h = relu(x W1_e + b1_e)  (h kept transposed) ----
                h_sb = hp.tile([128, HT, 512], bf16, tag="h_sb")
                for hh in range(2):
                    h_ps = [
                        psh.tile([128, 512], f32, tag="h", name=f"h_ps{m}")
                        for m in range(4)
                    ]
                    for m in range(4):
                        for k in range(KT):
                            nc.tensor.matmul(
                                h_ps[m][:, :W],
                                lhsT=w1t[e][hh][:, k, m * 128 : (m + 1) * 128],
                                rhs=xgb[:, k, :W],
                                start=(k == 0),
                                stop=(k == KT - 1),
                            )
                    n_scalar_relu = 3 if b2_zero else 2
                    for m in range(4):
                        j = hh * 4 + m
                        if m < n_scalar_relu:
                            nc.scalar.activation(
                                h_sb[:, j, :W],
                                h_ps[m][:, :W],
                                AF.Relu,
                                bias=b1_sb[:, e, j : j + 1],
                                scale=1.0,
                            )
                        else:
                            # relu(h + b1) on DVE: (in + b1) max 0
                            nc.vector.tensor_scalar(
                                h_sb[:, j, :W],
                                h_ps[m][:, :W],
                                b1_sb[:, e, j : j + 1],
                                0.0,
                                op0=ALU.add,
                                op1=ALU.max,
                            )

                # ---- y = h W2_e; emit gate * (y + b2_e) ----
                if not b2_zero:
                    # gb2[:, i, :] = gate_i * b2_e  (batched DVE, broadcast APs)
                    gb2 = gbp.tile([128, MAXW, D_OUT], f32, tag="gb2")
                    nc.vector.tensor_tensor(
                        gb2[:, :w, :],
                        b2_sb[:, e : e + 1, :].broadcast_to([128, w, D_OUT]),
                        g_sb[:, s0 : s0 + w, :].broadcast_to([128, w, D_OUT]),
                        op=ALU.mult,
                    )
                ot = outp.tile([128, MAXW, D_OUT], f32, tag="ot")
                for i in range(w):
                    y_ps = psy.tile([128, D_OUT], f32, tag="y")
                    for j in range(HT):
                        nc.tensor.matmul(
                            y_ps[:],
                            lhsT=h_sb[:, j, i * 128 : (i + 1) * 128],
                            rhs=w2t[e][:, j * D_OUT : (j + 1) * D_OUT],
                            start=(j == 0),
                            stop=(j == HT - 1),
                        )
                    if b2_zero:
                        # ot = gate * y (DVE, per-partition scalar)
                        nc.vector.tensor_scalar_mul(
                            ot[:, i, :], y_ps[:], g_sb[:, s0 + i, :]
                        )
                    else:
                        # gy = gate * y (scalar engine, per-partition scale)
                        gy = smp.tile([128, D_OUT], f32, tag="gy")
                        nc.scalar.activation(
                            gy[:], y_ps[:], AF.Copy, bias=0.0,
                            scale=g_sb[:, s0 + i, :],
                        )
                        nc.vector.tensor_add(ot[:, i, :], gy[:], gb2[:, i, :])
                nc.sync.dma_start(out=out[:, s0 : s0 + w, :], in_=ot[:, :w, :])

    nc.compile()
    return nc, NT


_NC_CACHE = {}


def _get_nc(tiles_key, b2_zero):
    key = (tiles_key, b2_zero)
    if key not in _NC_CACHE:
        _NC_CACHE[key] = build_dispatch(tiles_key, b2_zero=b2_zero)
    return _NC_CACHE[key]


def _route_and_prep(inputs):
    """Host-side routing/dispatch (same math as the reference router, fp32)
    + input staging for all cores."""
    import ml_dtypes

    x = np.asarray(inputs["x"], dtype=np.float32)
    spike = np.asarray(inputs["spike_indicators"], dtype=np.float32)
    Wr = np.asarray(inputs["Wr"], dtype=np.float32)
    br = np.asarray(inputs["br"], dtype=np.float32)
    W1 = np.asarray(inputs["W1"], dtype=np.float32)
    b1 = np.asarray(inputs["b1"], dtype=np.float32)
    W2 = np.asarray(inputs["W2"], dtype=np.float32)
    b2 = np.asarray(inputs["b2"], dtype=np.float32)
    Bn = x.shape[0]

    # router (fp32, same formula as reference)
    logits = x @ Wr + br
    adj = logits
    adj[:, 8:10] += spike.mean(axis=1, keepdims=True)
    top2 = np.argpartition(-adj, 2, axis=1)[:, :3]
    rows = np.arange(Bn)[:, None]
    ordsel = np.argsort(-adj[rows, top2], axis=1, kind="stable")
    top2 = top2[rows, ordsel][:, :2]
    # softmax probs of the top-2, normalized (matches reference numerics)
    m = adj.max(axis=1, keepdims=True)
    ez = np.exp(adj - m)
    p = ez / ez.sum(axis=1, keepdims=True)
    tp = p[rows, top2]  # [B, 2]
    gate2 = tp / (tp.sum(axis=1, keepdims=True) + 1e-9)

    # per-expert slot lists, padded so every core gets the same tile counts
    tiles_per_core = np.zeros(E, dtype=np.int64)
    sel_per_e = []
    for e in range(E):
        sel = np.nonzero((top2[:, 0] == e) | (top2[:, 1] == e))[0]
        sel_per_e.append(sel)
        tiles_per_core[e] = (len(sel) + 128 * N_CORES - 1) // (128 * N_CORES)
    NT = int(tiles_per_core.sum())
    TC = NT * 128

    # slot -> token maps per core, slot gates, and token -> flat-slot inverse
    tok_of_slot = np.zeros((N_CORES, TC), dtype=np.int64)
    gate_of_slot = np.zeros((N_CORES, TC), dtype=np.float32)
    pos_global = np.full((Bn, TOP_K), -1, dtype=np.int64)
    base = 0
    for e in range(E):
        sel = sel_per_e[e]
        if len(sel) == 0:
            continue
        per_core = int(tiles_per_core[e]) * 128
        q = np.arange(len(sel))
        core = q // per_core
        pos = base * 128 + (q % per_core)
        k_of = np.where(top2[sel, 0] == e, 0, 1)
        pos_global[sel, k_of] = core * TC + pos
        g_e = gate2[sel, k_of]
        for c in range(N_CORES):
            seg = slice(c * per_core, min((c + 1) * per_core, len(sel)))
            n = seg.stop - seg.start
            if n <= 0:
                break
            tok_of_slot[c, base * 128 : base * 128 + n] = sel[seg]
            gate_of_slot[c, base * 128 : base * 128 + n] = g_e[seg]
        base += int(tiles_per_core[e])
    assert (pos_global >= 0).all()

    # shared (replicated) tensors
    W1_t = np.ascontiguousarray(
        W1.reshape(E, KT, 128, HIDDEN).transpose(0, 2, 1, 3)
    ).astype(ml_dtypes.bfloat16)  # [E, 128, KT, HIDDEN]
    W2_t = np.ascontiguousarray(
        W2.reshape(E, HT, 128, D_OUT).transpose(0, 2, 1, 3).reshape(E, 128, HT * D_OUT)
    ).astype(ml_dtypes.bfloat16)
    b1_t = np.ascontiguousarray(b1.reshape(E, HT, 128).transpose(2, 0, 1))
    b2_t = np.ascontiguousarray(np.broadcast_to(b2[None, :, :], (128, E, D_OUT)))
    shared = {"W1": W1_t, "b1r": b1_t, "W2": W2_t, "b2r": b2_t}

    xT = np.ascontiguousarray(x.T).astype(ml_dtypes.bfloat16)  # [D_IN, B]
    in_maps = []
    for c in range(N_CORES):
        toks = tok_of_slot[c]
        xc = xT[:, toks]  # [512, TC] bf16
        xg = np.ascontiguousarray(xc.reshape(KT, 128, TC).transpose(1, 0, 2))
        gg = np.ascontiguousarray(gate_of_slot[c].reshape(NT, 128).T)[:, :, None]
        in_maps.append({"xg": xg, "gates": gg, **shared})
    b2_zero = not np.any(b2)
    return in_maps, pos_global, tuple(int(t) for t in tiles_per_core), NT, b2_zero


def kernel(**inputs) -> np.ndarray:
    in_maps, pos_global, tiles_key, NT, b2_zero = _route_and_prep(inputs)
    nc, _ = _get_nc(tiles_key, b2_zero)
    res = run_bass_kernel_spmd(nc, in_maps, core_ids=list(range(N_CORES)))
    Ycat = np.concatenate(
        [res.results[c]["out"].transpose(1, 0, 2).reshape(NT * 128, D_OUT)
         for c in range(N_CORES)],
        axis=0,
    )
    out = Ycat[pos_global[:, 0]] + Ycat[pos_global[:, 1]]
    return out.astype(np.float32)


def run_traced(tmpdir=None, **inputs):
    in_maps, pos_global, tiles_key, NT, b2_zero = _route_and_prep(inputs)
    nc, _ = _get_nc(tiles_key, b2_zero)
    return run_bass_kernel_spmd(
        nc, in_maps, core_ids=list(range(N_CORES)), trace=True, tmpdir=tmpdir
    )
